# revision 38
# baseline (speedup 1.0000x reference)
"""Cross-attention kernel for Trainium2, distributed over 8 NeuronCores.

Sharding: data-parallel over batch (4) x tensor-parallel over head groups (2).
Core c handles batch b = c//2, heads [4g, 4g+4) with g = c%2.

Key structural ideas (vs. a dense implementation):

* Host-side compaction. Masked queries (mask[b,i]=False) all produce the
  SAME output row: softmax over an all-masked row is uniform over all m+1
  positions, so out_i = (sum_j v_j + nv)/(m+1) @ Wo + bo — computed on the
  host. Masked context positions contribute exactly 0 after softmax. The
  device only sees the ~50% active queries and ~50% unmasked context
  columns (null token at column 0), cutting attention work ~4x. Padding
  to 128 multiples: pad queries are zero columns (output discarded); pad
  context columns are zeroed and excluded from softmax by a zero in the
  ones-column of the augmented v (so they add 0 to both numerator and
  denominator — no mask bias needed anywhere).

* bf16 matmul operands everywhere; PSUM accumulation stays fp32. PE runs
  1 cycle/row for bf16 vs 4 for fp32. Tolerance is 2e-2; bf16 lands ~6e-3.

* The attention inner loop is Act-engine-paced (exp is Act-only). Per
  (i-chunk, head-pair): S matmuls and exp run 2 j-tiles ahead of the
  attn@v matmuls (PSUM: 3 score bufs x 2 banks + 1 accumulator x 2 banks),
  so PE never blocks on the S->exp->av latency chain. PE slack inside the
  Act-paced loop is filled with the next i-chunk's q projection and the
  previous i-chunk's output projection.

* A burst of dummy PE matmuls at t=0 keeps the tensor engine busy while
  input DMAs stream, so the p-state ramp (full clock after 3us of
  continuous execution) completes before real work starts.

* Softmax denominator: v is augmented with a ones column (row 64 of each
  head's accumulation). 1/den row -> bf16, broadcast across partitions by
  a K=1 matmul into PSUM, one DVE multiply per head pair.
"""

import numpy as np
import ml_dtypes

import concourse.bass as bass
import concourse.tile as tile
from concourse import bacc, bass_utils, mybir

FP = mybir.dt.float32
BF = mybir.dt.bfloat16
AF = mybir.ActivationFunctionType
NPBF = ml_dtypes.bfloat16

B, N, M, DIM = 4, 2048, 2048, 512
HEADS, DH = 8, 64
INNER = HEADS * DH
G = 2          # head groups (tensor-parallel degree)
HG = 4         # heads per group
DG = HG * DH   # 256 dims per group
SCALE = 1.0 / np.sqrt(DH)  # 0.125
VW = DH + 1    # v columns per head incl. ones column (den row)

LAST_RESULTS = None
LAST_NC = None
_CACHE = {}


_SPLIT_SKIP = (
    "InstDrain", "InstUnconditionalBranch", "InstCall",
    "InstEventSemaphore", "InstRegisterMove", "InstDmaTrigger",
)


def _split_multi_waits(nc):
    """TRN2 TPB instruction structs accept only ONE sync wait in walrus
    codegen; extra waits assigned by the Tile scheduler are silently dropped
    from the NEFF, which races on hardware. Hoist all-but-one wait onto
    standalone same-engine InstEventSemaphore instructions (sequencer-only
    waits, the same mechanism the framework itself uses) placed immediately
    before the offending instruction."""
    valid = set(mybir.EngineType) - {mybir.EngineType.Unassigned}
    total = 0
    for bb in nc.m.functions[0].blocks:
        new_insts = []
        for ins in bb.instructions:
            si = ins.sync_info
            if (
                getattr(ins, "engine", None) in valid
                and type(ins).__name__ not in _SPLIT_SKIP
                and si is not None
                and si.on_wait
                and len(si.on_wait) > 1
            ):
                waits = list(si.on_wait)
                for w in waits[:-1]:
                    total += 1
                    ev = mybir.InstEventSemaphore(
                        name=f"evsplit{total}_{ins.name}", ins=[], outs=[])
                    ev.engine = ins.engine
                    ev.sync_info = mybir.SyncInfo(on_wait=[w], on_update=[])
                    nc.inst_map[ev.name] = ev
                    new_insts.append(ev)
                si.on_wait = waits[-1:]
            new_insts.append(ins)
        bb.instructions = new_insts
    return total


def _chunks(total):
    """Split total (a multiple of 128) into <=512-sized 128-multiples,
    descending, each >=256 where possible (256 is the fp-fast-path floor
    for PE moving dims; a smaller final chunk also shrinks the kernel's
    serial tail)."""
    out, off, rem = [], 0, total
    while rem:
        take = min(rem, 512)
        if rem - take == 128:
            take = 384
        out.append((off, take))
        off += take
        rem -= take
    return out


def _build(npi, npj):
    nc = bacc.Bacc("TRN2", debug=False, num_devices=8, enable_partition_id=False)
    d = {}

    def inp(name, shape, dt):
        d[name] = nc.dram_tensor(name, shape, dt, kind="ExternalInput").ap()

    jtc = npj // 128
    inp("xT", [DIM, npi], BF)
    inp("cxT", [DIM, npj], BF)
    inp("wq", [DIM, DG], BF)
    inp("wk", [DIM, DG], BF)
    inp("wv", [DIM, DG], BF)
    inp("wo", [DG, DIM], BF)
    inp("vones", [128, jtc * HG], BF)  # 1 for valid j rows (incl null), 0 pads
    inp("nk", [128, 1], FP)            # null_key tiled x2
    inp("nv", [1, DG], BF)             # null_value tiled x4
    d["out"] = nc.dram_tensor("out", [npi, DIM], FP, kind="ExternalOutput").ap()

    with tile.TileContext(nc) as tc:
        _body(tc, d, npi, npj)
    nc.compile()
    return nc


def _body(tc, d, npi, npj):
    nc = tc.nc
    jtc = npj // 128
    ichunks = _chunks(npi)
    jchunks = _chunks(npj)

    with (
        tc.tile_pool(name="consts", bufs=1) as consts,
        tc.tile_pool(name="big", bufs=1) as big,
        tc.tile_pool(name="spool", bufs=4) as spool,
        tc.tile_pool(name="fop", bufs=2) as fop,
        tc.tile_pool(name="dpool", bufs=2) as dpool,
        tc.tile_pool(name="sp", bufs=2, space="PSUM") as sp_ps,
        tc.tile_pool(name="acc", bufs=2, space="PSUM") as acc_ps,
    ):
        # ---- inputs. One whole tile per DMA (sliced DMA writes into a
        # shared tile mis-sync at the NEFF level — see module docstring);
        # x/ctx are split into per-chunk tiles so each projection chunk can
        # start as soon as its own transfer lands. Ordered so the k
        # projection (first consumer after warmup) unblocks earliest.
        wk = consts.tile([128, 4, DG], BF)
        nc.sync.dma_start(wk[:], d["wk"].rearrange("(c p) d -> p c d", p=128))
        cxSrc = d["cxT"].rearrange("(c p) j -> p c j", p=128)
        xSrc = d["xT"].rearrange("(c p) i -> p c i", p=128)
        cxTt, xTt = [], []
        cxTt.append(big.tile([128, 4, jchunks[0][1]], BF, name="cxT0"))
        nc.sync.dma_start(cxTt[0][:], cxSrc[:, :, 0:jchunks[0][1]])
        wq = consts.tile([128, 4, DG], BF)
        nc.sync.dma_start(wq[:], d["wq"].rearrange("(c p) d -> p c d", p=128))
        xTt.append(big.tile([128, 4, ichunks[0][1]], BF, name="xT0"))
        nc.sync.dma_start(xTt[0][:], xSrc[:, :, 0:ichunks[0][1]])
        for c, (off, cs) in enumerate(jchunks[1:], 1):
            t = big.tile([128, 4, cs], BF, name=f"cxT{c}")
            nc.sync.dma_start(t[:], cxSrc[:, :, off:off + cs])
            cxTt.append(t)
        for c, (off, cs) in enumerate(ichunks[1:], 1):
            t = big.tile([128, 4, cs], BF, name=f"xT{c}")
            nc.sync.dma_start(t[:], xSrc[:, :, off:off + cs])
            xTt.append(t)
        wv = consts.tile([128, 4, DG], BF)
        nc.sync.dma_start(wv[:], d["wv"].rearrange("(c p) d -> p c d", p=128))
        wo = consts.tile([128, 2, DIM], BF)
        nc.sync.dma_start(wo[:], d["wo"].rearrange("(c p) o -> p c o", p=128))
        nk = consts.tile([128, 1], FP)
        nc.sync.dma_start(nk[:], d["nk"])

        def cx_loc(j0):
            """Map a global j column offset to (chunk tile, local offset)."""
            for c, (off, cs) in enumerate(jchunks):
                if j0 < off + cs:
                    return cxTt[c], j0 - off
            raise AssertionError(j0)

        qT = big.tile([128, 2, npi], BF)
        kT = big.tile([128, 2, npj], BF)
        vsb = big.tile([128, jtc, HG, VW], BF)
        Osb = big.tile([128, 2, npi], BF)
        # vones/nv bounce through whole tiles + engine copies: sliced DMA
        # writes into vsb are not reliably ordered against its readers
        vot = consts.tile([128, jtc * HG], BF)
        nc.sync.dma_start(vot[:], d["vones"])
        nc.vector.tensor_copy(
            vsb[:, :, :, DH:VW],
            vot[:].rearrange("p (j h o) -> p j h o", h=HG, o=1))
        nvt = consts.tile([1, DG], BF)
        nc.sync.dma_start(nvt[:], d["nv"])

        ones_row = consts.tile([1, DH], BF)
        nc.vector.memset(ones_row[:], 1.0)

        # ---- PE warmup: dummy matmuls keep the p-state ramp going while
        # the input DMAs stream (ramp hits full clock after 3us busy)
        wsrc = consts.tile([128, 256], BF)
        nc.vector.memset(wsrc[:], 0.5)
        wps = sp_ps.tile([128, 2, 512], FP, tag="sp", name="warm")
        for i in range(8):
            nc.tensor.matmul(wps[:, i % 2, 0:256], wsrc[:, 0:128], wsrc[:],
                             start=True, stop=True)

        def qproj(ci):
            off, cs = ichunks[ci]
            ps = sp_ps.tile([128, 2, 512], FP, tag="sp", name=f"psq{off}")
            for dc in range(2):
                for cc in range(4):
                    nc.tensor.matmul(
                        ps[:, dc, :cs],
                        wq[:, cc, dc * 128:(dc + 1) * 128],
                        xTt[ci][:, cc, :cs],
                        start=(cc == 0), stop=(cc == 3),
                    )
            nc.scalar.activation(qT[:, :, off:off + cs], ps[:, :, :cs], AF.Tanh)

        def outproj(ci):
            off, cs = ichunks[ci]
            for t in range(cs // 128):
                it = off // 128 + t
                pf = sp_ps.tile([128, 2, 512], FP, tag="sp", name=f"pf{it}")
                for dc in range(2):
                    nc.tensor.matmul(
                        pf[:, 0, :],
                        Osb[:, dc, it * 128:(it + 1) * 128],
                        wo[:, dc, :],
                        start=(dc == 0), stop=(dc == 1),
                    )
                fo = fop.tile([128, 512], FP, tag="fo", name=f"fo{it}")
                nc.vector.tensor_copy(fo[:], pf[:, 0, :])
                nc.sync.dma_start(d["out"][it * 128:(it + 1) * 128, :], fo[:])

        # ---- projections needed before attention: all k, q chunk 0, all v
        for c, (off, cs) in enumerate(jchunks):
            ps = sp_ps.tile([128, 2, 512], FP, tag="sp", name=f"psk{off}")
            for dc in range(2):
                for cc in range(4):
                    nc.tensor.matmul(
                        ps[:, dc, :cs],
                        wk[:, cc, dc * 128:(dc + 1) * 128],
                        cxTt[c][:, cc, :cs],
                        start=(cc == 0), stop=(cc == 3),
                    )
            nc.scalar.activation(kT[:, :, off:off + cs], ps[:, :, :cs], AF.Tanh)
        for dc in range(2):
            nc.scalar.activation(kT[:, dc, 0:1], nk[:], AF.Tanh)

        qproj(0)

        for jt0 in range(0, jtc, 2):
            ps = sp_ps.tile([128, 2, 512], FP, tag="sp", name=f"psv{jt0}")
            for s in range(2):
                jt = jt0 + s
                if jt >= jtc:
                    break
                src, loc = cx_loc(jt * 128)
                for cc in range(4):
                    nc.tensor.matmul(
                        ps[:, s, 0:DG],
                        src[:, cc, loc:loc + 128],
                        wv[:, cc, :],
                        start=(cc == 0), stop=(cc == 3),
                    )
                nc.vector.tensor_copy(
                    vsb[:, jt, :, 0:DH],
                    ps[:, s, 0:DG].rearrange("p (h e) -> p h e", h=HG),
                )
        # null token value at j=0 — must land after the vproj copy of tile 0
        nc.vector.tensor_copy(vsb[0:1, 0, :, 0:DH],
                              nvt[:].rearrange("a (h e) -> a h e", h=HG))

        # ---- attention: one pipelined stream over (i-chunk, head-pair)
        # segments x j tiles. S matmuls + exp run 2 j-tiles ahead of the
        # attn@v matmuls, ACROSS segment boundaries, so the PE never sits
        # through the Act engine's exp drain at a segment's tail. Each
        # segment's denominator division is emitted right after its last
        # attn@v (i.e. inside the next segment's stream); PE slack inside
        # the Act-paced loop is filled with q/out projections.
        nic = len(ichunks)
        segs = [(ci, hp) for ci in range(nic) for hp in range(2)]
        po_of = {}

        def emit_av(item):
            ssb, jt, ci, hp = item
            off, cs = ichunks[ci]
            if jt == 0:  # lazily created so pool-buffer order == use order
                po_of[(ci, hp)] = acc_ps.tile([128, 2, 512], FP, tag="po",
                                              name=f"po{ci}{hp}")
            po2 = po_of[(ci, hp)]
            for hh in range(2):
                nc.tensor.matmul(
                    po2[0:VW, hh, :cs],
                    vsb[:, jt, 2 * hp + hh, :],
                    ssb[:, hh, :cs],
                    start=(jt == 0), stop=(jt == jtc - 1),
                )
            if jt == jtc - 1:
                den_div(ci, hp)

        def den_div(ci, hp):
            # divide by denominator (row DH of each head's po2)
            off, cs = ichunks[ci]
            po2 = po_of[(ci, hp)]
            posb = dpool.tile([128, 2, 512], FP, tag="posb")
            nc.vector.tensor_copy(posb[0:VW, :, :cs], po2[0:VW, :, :cs])
            den_r = dpool.tile([1, 2, 512], BF, tag="den")
            with nc.allow_low_precision(reason="bf16 1/den; tol 2e-2"):
                nc.vector.reciprocal(den_r[:, :, :cs], posb[DH:VW, :, :cs])
            pr = sp_ps.tile([128, 2, 512], FP, tag="sp", name=f"pr{ci}{hp}")
            for s in range(2):
                nc.tensor.matmul(pr[0:DH, s, :cs], ones_row[:],
                                 den_r[0:1, s, :cs], start=True, stop=True)
            tmpo = dpool.tile([64, 2, 512], BF, tag="tmpo")
            nc.vector.tensor_mul(tmpo[:, :, :cs], posb[0:DH, :, :cs],
                                 pr[0:DH, :, :cs])
            for s in range(2):
                nc.sync.dma_start(
                    Osb[64 * s:64 * s + DH, hp, off:off + cs],
                    tmpo[:, s, :cs])

        pend = []
        for ci, hp in segs:
            off, cs = ichunks[ci]
            for jt in range(jtc):
                sps = sp_ps.tile([128, 2, 512], FP, tag="sp",
                                 name=f"s{ci}_{hp}_{jt}")
                for hh in range(2):
                    nc.tensor.matmul(
                        sps[:, hh, :cs],
                        kT[64 * hh:64 * hh + DH, hp, jt * 128:(jt + 1) * 128],
                        qT[64 * hh:64 * hh + DH, hp, off:off + cs],
                        start=True, stop=True,
                    )
                ssb = spool.tile([128, 2, 512], BF, tag="s",
                                 name=f"e{ci}_{hp}_{jt}")
                nc.scalar.activation(ssb[:, :, :cs], sps[:, :, :cs],
                                     AF.Exp, scale=float(SCALE))
                pend.append((ssb, jt, ci, hp))
                if len(pend) > 2:
                    emit_av(pend.pop(0))
            # PE filler between segments while Act drains pending exps
            if hp == 0 and ci + 1 < nic:
                qproj(ci + 1)
            if hp == 1 and ci > 0:
                outproj(ci - 1)
        while pend:
            emit_av(pend.pop(0))
        outproj(nic - 1)


def _core_inputs(inputs, core, npi, npj, idx_i, idx_j):
    b, g = core // 2, core % 2
    x = np.asarray(inputs["x"], np.float32)
    context = np.asarray(inputs["context"], np.float32)
    Wq = np.asarray(inputs["Wq"], np.float32)
    Wkv = np.asarray(inputs["Wkv"], np.float32)
    Wo = np.asarray(inputs["Wo"], np.float32)
    null_key = np.asarray(inputs["null_key"], np.float32)
    null_value = np.asarray(inputs["null_value"], np.float32)

    ii, jj = idx_i[b], idx_j[b]
    jtc = npj // 128

    xT = np.zeros((DIM, npi), NPBF)
    xT[:, :len(ii)] = x[b][ii].T
    cxT = np.zeros((DIM, npj), NPBF)
    cxT[:, 1:1 + len(jj)] = context[b][jj].T

    # validity of each j row (incl. null at 0), replicated per head
    valid = (np.arange(npj) < 1 + len(jj)).astype(np.float32)
    vones = np.repeat(valid.reshape(jtc, 128).T[:, :, None], HG, axis=2)

    gs = slice(g * DG, (g + 1) * DG)
    return {
        "xT": xT,
        "cxT": cxT,
        "wq": Wq[:, gs].astype(NPBF),
        "wk": Wkv[:, gs].astype(NPBF),
        "wv": Wkv[:, DIM + g * DG: DIM + (g + 1) * DG].astype(NPBF),
        "wo": Wo[gs, :].astype(NPBF),
        "vones": np.ascontiguousarray(vones.reshape(128, jtc * HG)).astype(NPBF),
        "nk": np.ascontiguousarray(np.tile(null_key, 2).reshape(128, 1)),
        "nv": np.tile(null_value, HG).reshape(1, DG).astype(NPBF),
    }


def kernel(x, context, mask, context_mask, Wq, Wkv, Wo, bo, null_key, null_value):
    global LAST_RESULTS, LAST_NC
    inputs = {
        "x": x, "context": context, "mask": mask, "context_mask": context_mask,
        "Wq": Wq, "Wkv": Wkv, "Wo": Wo, "bo": bo,
        "null_key": null_key, "null_value": null_value,
    }
    mask_np = np.asarray(mask, bool)
    cm_np = np.asarray(context_mask, bool)
    idx_i = [np.nonzero(mask_np[b])[0] for b in range(B)]
    idx_j = [np.nonzero(cm_np[b])[0] for b in range(B)]
    npi = max(128, -(-max(len(ii) for ii in idx_i) // 128) * 128)
    npj = max(128, -(-max(1 + len(jj) for jj in idx_j) // 128) * 128)

    key = (npi, npj)
    if key not in _CACHE:
        _CACHE[key] = _build(npi, npj)
    nc = _CACHE[key]
    LAST_NC = nc

    in_maps = [_core_inputs(inputs, core, npi, npj, idx_i, idx_j)
               for core in range(8)]
    res = bass_utils.run_bass_kernel_spmd(nc, in_maps, core_ids=list(range(8)))
    LAST_RESULTS = res

    Wkv_np = np.asarray(Wkv, np.float32)
    Wo_np = np.asarray(Wo, np.float32)
    bo_np = np.asarray(bo, np.float32)
    nv_full = np.tile(np.asarray(null_value, np.float32), HEADS)

    out = np.empty((B, N, DIM), np.float32)
    for b in range(B):
        nact = len(idx_i[b])
        if nact:
            s = (res.results[2 * b]["out"][:nact]
                 + res.results[2 * b + 1]["out"][:nact] + bo_np)
            out[b][idx_i[b]] = s
        # masked queries attend uniformly over ALL m+1 positions
        vsum = np.asarray(context[b], np.float32).sum(0) @ Wkv_np[:, INNER:]
        urow = (vsum + nv_full) / (M + 1) @ Wo_np + bo_np
        out[b][~mask_np[b]] = urow
    return out


# revision 41
# speedup vs baseline: 1.1009x; 1.1009x over previous
"""Cross-attention kernel for Trainium2, distributed over 8 NeuronCores.

Sharding: data-parallel over batch (4) x tensor-parallel over head groups (2).
Core c handles batch b = c//2, heads [4g, 4g+4) with g = c%2.

Key structural ideas (vs. a dense implementation):

* Host-side compaction. Masked queries (mask[b,i]=False) all produce the
  SAME output row: softmax over an all-masked row is uniform over all m+1
  positions, so out_i = (sum_j v_j + nv)/(m+1) @ Wo + bo — computed on the
  host. Masked context positions contribute exactly 0 after softmax. The
  device only sees the ~50% active queries and ~50% unmasked context
  columns (null token at column 0), cutting attention work ~4x. Padding
  to 128 multiples: pad queries are zero columns (output discarded); pad
  context columns are zeroed and excluded from softmax by a zero in the
  ones-column of the augmented v (so they add 0 to both numerator and
  denominator — no mask bias needed anywhere).

* bf16 matmul operands everywhere; PSUM accumulation stays fp32. PE runs
  1 cycle/row for bf16 vs 4 for fp32. Tolerance is 2e-2; bf16 lands ~6e-3.

* The attention inner loop is Act-engine-paced (exp is Act-only). Per
  (i-chunk, head-pair): S matmuls and exp run 2 j-tiles ahead of the
  attn@v matmuls (PSUM: 3 score bufs x 2 banks + 1 accumulator x 2 banks),
  so PE never blocks on the S->exp->av latency chain. PE slack inside the
  Act-paced loop is filled with the next i-chunk's q projection and the
  previous i-chunk's output projection.

* A burst of dummy PE matmuls at t=0 keeps the tensor engine busy while
  input DMAs stream, so the p-state ramp (full clock after 3us of
  continuous execution) completes before real work starts.

* Softmax denominator: v is augmented with a ones column (row 64 of each
  head's accumulation). 1/den row -> bf16, broadcast across partitions by
  a K=1 matmul into PSUM, one DVE multiply per head pair.
"""

import numpy as np
import ml_dtypes

import concourse.bass as bass
import concourse.tile as tile
from concourse import bacc, bass_utils, mybir

FP = mybir.dt.float32
BF = mybir.dt.bfloat16
AF = mybir.ActivationFunctionType
NPBF = ml_dtypes.bfloat16

B, N, M, DIM = 4, 2048, 2048, 512
HEADS, DH = 8, 64
INNER = HEADS * DH
G = 2          # head groups (tensor-parallel degree)
HG = 4         # heads per group
DG = HG * DH   # 256 dims per group
SCALE = 1.0 / np.sqrt(DH)  # 0.125
VW = DH + 1    # v columns per head incl. ones column (den row)

LAST_RESULTS = None
LAST_NC = None
_CACHE = {}


_SPLIT_SKIP = (
    "InstDrain", "InstUnconditionalBranch", "InstCall",
    "InstEventSemaphore", "InstRegisterMove", "InstDmaTrigger",
)


def _split_multi_waits(nc):
    """TRN2 TPB instruction structs accept only ONE sync wait in walrus
    codegen; extra waits assigned by the Tile scheduler are silently dropped
    from the NEFF, which races on hardware. Hoist all-but-one wait onto
    standalone same-engine InstEventSemaphore instructions (sequencer-only
    waits, the same mechanism the framework itself uses) placed immediately
    before the offending instruction."""
    valid = set(mybir.EngineType) - {mybir.EngineType.Unassigned}
    total = 0
    for bb in nc.m.functions[0].blocks:
        new_insts = []
        for ins in bb.instructions:
            si = ins.sync_info
            if (
                getattr(ins, "engine", None) in valid
                and type(ins).__name__ not in _SPLIT_SKIP
                and si is not None
                and si.on_wait
                and len(si.on_wait) > 1
            ):
                waits = list(si.on_wait)
                for w in waits[:-1]:
                    total += 1
                    ev = mybir.InstEventSemaphore(
                        name=f"evsplit{total}_{ins.name}", ins=[], outs=[])
                    ev.engine = ins.engine
                    ev.sync_info = mybir.SyncInfo(on_wait=[w], on_update=[])
                    nc.inst_map[ev.name] = ev
                    new_insts.append(ev)
                si.on_wait = waits[-1:]
            new_insts.append(ins)
        bb.instructions = new_insts
    return total


def _chunks(total):
    """Split total (a multiple of 128) into <=512-sized 128-multiples,
    descending, each >=256 where possible (256 is the fp-fast-path floor
    for PE moving dims; a smaller final chunk also shrinks the kernel's
    serial tail)."""
    out, off, rem = [], 0, total
    while rem:
        take = min(rem, 512)
        if rem - take == 128:
            take = 384
        out.append((off, take))
        off += take
        rem -= take
    return out


def _build(npi, npj):
    nc = bacc.Bacc("TRN2", debug=False, num_devices=8, enable_partition_id=False)
    d = {}

    def inp(name, shape, dt):
        d[name] = nc.dram_tensor(name, shape, dt, kind="ExternalInput").ap()

    jtc = npj // 128
    inp("xT", [DIM, npi], BF)
    inp("cxT", [DIM, npj], BF)
    inp("wq", [DIM, DG], BF)
    inp("wk", [DIM, DG], BF)
    inp("wv", [DIM, DG], BF)
    inp("wo", [DG, DIM], BF)
    inp("vones", [128, jtc * HG], BF)  # 1 for valid j rows (incl null), 0 pads
    inp("nk", [128, 1], FP)            # null_key tiled x2
    inp("nv", [1, DG], BF)             # null_value tiled x4
    d["out"] = nc.dram_tensor("out", [npi, DIM], FP, kind="ExternalOutput").ap()

    with tile.TileContext(nc) as tc:
        _body(tc, d, npi, npj)
    nc.compile()
    return nc


def _body(tc, d, npi, npj):
    nc = tc.nc
    jtc = npj // 128
    ichunks = _chunks(npi)
    jchunks = _chunks(npj)

    with (
        tc.tile_pool(name="consts", bufs=1) as consts,
        tc.tile_pool(name="big", bufs=1) as big,
        tc.tile_pool(name="spool", bufs=6) as spool,
        tc.tile_pool(name="fop", bufs=2) as fop,
        tc.tile_pool(name="dpool", bufs=2) as dpool,
        tc.tile_pool(name="sp", bufs=3, space="PSUM") as sp_ps,
        tc.tile_pool(name="acc", bufs=1, space="PSUM") as acc_ps,
    ):
        # ---- inputs. One whole tile per DMA (sliced DMA writes into a
        # shared tile mis-sync at the NEFF level — see module docstring);
        # x/ctx are split into per-chunk tiles so each projection chunk can
        # start as soon as its own transfer lands. Ordered so the k
        # projection (first consumer after warmup) unblocks earliest.
        wk = consts.tile([128, 4, DG], BF)
        nc.sync.dma_start(wk[:], d["wk"].rearrange("(c p) d -> p c d", p=128))
        cxSrc = d["cxT"].rearrange("(c p) j -> p c j", p=128)
        xSrc = d["xT"].rearrange("(c p) i -> p c i", p=128)
        cxTt, xTt = [], []
        cxTt.append(big.tile([128, 4, jchunks[0][1]], BF, name="cxT0"))
        nc.sync.dma_start(cxTt[0][:], cxSrc[:, :, 0:jchunks[0][1]])
        wq = consts.tile([128, 4, DG], BF)
        nc.sync.dma_start(wq[:], d["wq"].rearrange("(c p) d -> p c d", p=128))
        xTt.append(big.tile([128, 4, ichunks[0][1]], BF, name="xT0"))
        nc.sync.dma_start(xTt[0][:], xSrc[:, :, 0:ichunks[0][1]])
        for c, (off, cs) in enumerate(jchunks[1:], 1):
            t = big.tile([128, 4, cs], BF, name=f"cxT{c}")
            nc.sync.dma_start(t[:], cxSrc[:, :, off:off + cs])
            cxTt.append(t)
        for c, (off, cs) in enumerate(ichunks[1:], 1):
            t = big.tile([128, 4, cs], BF, name=f"xT{c}")
            nc.sync.dma_start(t[:], xSrc[:, :, off:off + cs])
            xTt.append(t)
        wv = consts.tile([128, 4, DG], BF)
        nc.sync.dma_start(wv[:], d["wv"].rearrange("(c p) d -> p c d", p=128))
        wo = consts.tile([128, 2, DIM], BF)
        nc.sync.dma_start(wo[:], d["wo"].rearrange("(c p) o -> p c o", p=128))
        nk = consts.tile([128, 1], FP)
        nc.sync.dma_start(nk[:], d["nk"])

        def cx_loc(j0):
            """Map a global j column offset to (chunk tile, local offset)."""
            for c, (off, cs) in enumerate(jchunks):
                if j0 < off + cs:
                    return cxTt[c], j0 - off
            raise AssertionError(j0)

        qT = big.tile([128, 2, npi], BF)
        kT = big.tile([128, 2, npj], BF)
        vsb = big.tile([128, jtc, HG, VW], BF)
        Osb = big.tile([128, 2, npi], BF)
        # vones/nv bounce through whole tiles + engine copies: sliced DMA
        # writes into vsb are not reliably ordered against its readers
        vot = consts.tile([128, jtc * HG], BF)
        nc.sync.dma_start(vot[:], d["vones"])
        nc.vector.tensor_copy(
            vsb[:, :, :, DH:VW],
            vot[:].rearrange("p (j h o) -> p j h o", h=HG, o=1))
        nvt = consts.tile([1, DG], BF)
        nc.sync.dma_start(nvt[:], d["nv"])

        ones_row = consts.tile([1, DH], BF)
        nc.vector.memset(ones_row[:], 1.0)

        # ---- PE warmup: dummy matmuls keep the p-state ramp going while
        # the input DMAs stream (ramp hits full clock after 3us busy)
        wsrc = consts.tile([128, 256], BF)
        nc.vector.memset(wsrc[:], 0.5)
        wps = sp_ps.tile([128, 2, 512], FP, tag="sp", name="warm")
        for i in range(8):
            nc.tensor.matmul(wps[:, i % 2, 0:256], wsrc[:, 0:128], wsrc[:],
                             start=True, stop=True)

        def qproj(ci):
            off, cs = ichunks[ci]
            ps = sp_ps.tile([128, 2, 512], FP, tag="sp", name=f"psq{off}")
            for dc in range(2):
                for cc in range(4):
                    nc.tensor.matmul(
                        ps[:, dc, :cs],
                        wq[:, cc, dc * 128:(dc + 1) * 128],
                        xTt[ci][:, cc, :cs],
                        start=(cc == 0), stop=(cc == 3),
                    )
            nc.scalar.activation(qT[:, :, off:off + cs], ps[:, :, :cs], AF.Tanh)

        def outproj(ci):
            off, cs = ichunks[ci]
            for t in range(cs // 128):
                it = off // 128 + t
                pf = sp_ps.tile([128, 2, 512], FP, tag="sp", name=f"pf{it}")
                for dc in range(2):
                    nc.tensor.matmul(
                        pf[:, 0, :],
                        Osb[:, dc, it * 128:(it + 1) * 128],
                        wo[:, dc, :],
                        start=(dc == 0), stop=(dc == 1),
                    )
                fo = fop.tile([128, 512], FP, tag="fo", name=f"fo{it}")
                nc.vector.tensor_copy(fo[:], pf[:, 0, :])
                nc.sync.dma_start(d["out"][it * 128:(it + 1) * 128, :], fo[:])

        # ---- projections needed before attention: all k, q chunk 0, all v
        for c, (off, cs) in enumerate(jchunks):
            ps = sp_ps.tile([128, 2, 512], FP, tag="sp", name=f"psk{off}")
            for dc in range(2):
                for cc in range(4):
                    nc.tensor.matmul(
                        ps[:, dc, :cs],
                        wk[:, cc, dc * 128:(dc + 1) * 128],
                        cxTt[c][:, cc, :cs],
                        start=(cc == 0), stop=(cc == 3),
                    )
            nc.scalar.activation(kT[:, :, off:off + cs], ps[:, :, :cs], AF.Tanh)
        for dc in range(2):
            nc.scalar.activation(kT[:, dc, 0:1], nk[:], AF.Tanh)

        qproj(0)

        for jt0 in range(0, jtc, 2):
            ps = sp_ps.tile([128, 2, 512], FP, tag="sp", name=f"psv{jt0}")
            for s in range(2):
                jt = jt0 + s
                if jt >= jtc:
                    break
                src, loc = cx_loc(jt * 128)
                for cc in range(4):
                    nc.tensor.matmul(
                        ps[:, s, 0:DG],
                        src[:, cc, loc:loc + 128],
                        wv[:, cc, :],
                        start=(cc == 0), stop=(cc == 3),
                    )
                nc.vector.tensor_copy(
                    vsb[:, jt, :, 0:DH],
                    ps[:, s, 0:DG].rearrange("p (h e) -> p h e", h=HG),
                )
        # null token value at j=0 — must land after the vproj copy of tile 0
        nc.vector.tensor_copy(vsb[0:1, 0, :, 0:DH],
                              nvt[:].rearrange("a (h e) -> a h e", h=HG))

        # ---- attention: one pipelined stream over (i-chunk, head-pair)
        # segments x j tiles. S matmuls + exp run 2 j-tiles ahead of the
        # attn@v matmuls, ACROSS segment boundaries, so the PE never sits
        # through the Act engine's exp drain at a segment's tail. Each
        # segment's denominator division is emitted right after its last
        # attn@v (i.e. inside the next segment's stream); PE slack inside
        # the Act-paced loop is filled with q/out projections.
        nic = len(ichunks)
        segs = [(ci, hp) for ci in range(nic) for hp in range(2)]
        po_of = {}

        def emit_av(item):
            ssb, jt, ci, hp = item
            off, cs = ichunks[ci]
            if jt == 0:  # lazily created so pool-buffer order == use order
                po_of[(ci, hp)] = acc_ps.tile([128, 2, 512], FP, tag="po",
                                              name=f"po{ci}{hp}")
            po2 = po_of[(ci, hp)]
            for hh in range(2):
                nc.tensor.matmul(
                    po2[0:VW, hh, :cs],
                    vsb[:, jt, 2 * hp + hh, :],
                    ssb[:, hh, :cs],
                    start=(jt == 0), stop=(jt == jtc - 1),
                )
            if jt == jtc - 1:
                den_div(ci, hp)

        def den_div(ci, hp):
            # divide by denominator (row DH of each head's po2)
            off, cs = ichunks[ci]
            po2 = po_of[(ci, hp)]
            posb = dpool.tile([128, 2, 512], FP, tag="posb")
            nc.vector.tensor_copy(posb[0:VW, :, :cs], po2[0:VW, :, :cs])
            den_r = dpool.tile([1, 2, 512], BF, tag="den")
            with nc.allow_low_precision(reason="bf16 1/den; tol 2e-2"):
                nc.vector.reciprocal(den_r[:, :, :cs], posb[DH:VW, :, :cs])
            pr = sp_ps.tile([128, 2, 512], FP, tag="sp", name=f"pr{ci}{hp}")
            for s in range(2):
                nc.tensor.matmul(pr[0:DH, s, :cs], ones_row[:],
                                 den_r[0:1, s, :cs], start=True, stop=True)
            tmpo = dpool.tile([64, 2, 512], BF, tag="tmpo")
            nc.vector.tensor_mul(tmpo[:, :, :cs], posb[0:DH, :, :cs],
                                 pr[0:DH, :, :cs])
            for s in range(2):
                nc.sync.dma_start(
                    Osb[64 * s:64 * s + DH, hp, off:off + cs],
                    tmpo[:, s, :cs])

        pend = []
        for ci, hp in segs:
            off, cs = ichunks[ci]
            for jt in range(jtc):
                sps = sp_ps.tile([128, 2, 512], FP, tag="sp",
                                 name=f"s{ci}_{hp}_{jt}")
                for hh in range(2):
                    nc.tensor.matmul(
                        sps[:, hh, :cs],
                        kT[64 * hh:64 * hh + DH, hp, jt * 128:(jt + 1) * 128],
                        qT[64 * hh:64 * hh + DH, hp, off:off + cs],
                        start=True, stop=True,
                    )
                ssb = spool.tile([128, 2, 512], BF, tag="s",
                                 name=f"e{ci}_{hp}_{jt}")
                nc.scalar.activation(ssb[:, :, :cs], sps[:, :, :cs],
                                     AF.Exp, scale=float(SCALE))
                pend.append((ssb, jt, ci, hp))
                if len(pend) > 3:
                    emit_av(pend.pop(0))
            # PE filler between segments while Act drains pending exps
            if hp == 0 and ci + 1 < nic:
                qproj(ci + 1)
            if hp == 1 and ci > 0:
                outproj(ci - 1)
        while pend:
            emit_av(pend.pop(0))
        outproj(nic - 1)


def _core_inputs(inputs, core, npi, npj, idx_i, idx_j):
    b, g = core // 2, core % 2
    x = np.asarray(inputs["x"], np.float32)
    context = np.asarray(inputs["context"], np.float32)
    Wq = np.asarray(inputs["Wq"], np.float32)
    Wkv = np.asarray(inputs["Wkv"], np.float32)
    Wo = np.asarray(inputs["Wo"], np.float32)
    null_key = np.asarray(inputs["null_key"], np.float32)
    null_value = np.asarray(inputs["null_value"], np.float32)

    ii, jj = idx_i[b], idx_j[b]
    jtc = npj // 128

    xT = np.zeros((DIM, npi), NPBF)
    xT[:, :len(ii)] = x[b][ii].T
    cxT = np.zeros((DIM, npj), NPBF)
    cxT[:, 1:1 + len(jj)] = context[b][jj].T

    # validity of each j row (incl. null at 0), replicated per head
    valid = (np.arange(npj) < 1 + len(jj)).astype(np.float32)
    vones = np.repeat(valid.reshape(jtc, 128).T[:, :, None], HG, axis=2)

    gs = slice(g * DG, (g + 1) * DG)
    return {
        "xT": xT,
        "cxT": cxT,
        "wq": Wq[:, gs].astype(NPBF),
        "wk": Wkv[:, gs].astype(NPBF),
        "wv": Wkv[:, DIM + g * DG: DIM + (g + 1) * DG].astype(NPBF),
        "wo": Wo[gs, :].astype(NPBF),
        "vones": np.ascontiguousarray(vones.reshape(128, jtc * HG)).astype(NPBF),
        "nk": np.ascontiguousarray(np.tile(null_key, 2).reshape(128, 1)),
        "nv": np.tile(null_value, HG).reshape(1, DG).astype(NPBF),
    }


def kernel(x, context, mask, context_mask, Wq, Wkv, Wo, bo, null_key, null_value):
    global LAST_RESULTS, LAST_NC
    inputs = {
        "x": x, "context": context, "mask": mask, "context_mask": context_mask,
        "Wq": Wq, "Wkv": Wkv, "Wo": Wo, "bo": bo,
        "null_key": null_key, "null_value": null_value,
    }
    mask_np = np.asarray(mask, bool)
    cm_np = np.asarray(context_mask, bool)
    idx_i = [np.nonzero(mask_np[b])[0] for b in range(B)]
    idx_j = [np.nonzero(cm_np[b])[0] for b in range(B)]
    npi = max(128, -(-max(len(ii) for ii in idx_i) // 128) * 128)
    npj = max(128, -(-max(1 + len(jj) for jj in idx_j) // 128) * 128)

    key = (npi, npj)
    if key not in _CACHE:
        _CACHE[key] = _build(npi, npj)
    nc = _CACHE[key]
    LAST_NC = nc

    in_maps = [_core_inputs(inputs, core, npi, npj, idx_i, idx_j)
               for core in range(8)]
    res = bass_utils.run_bass_kernel_spmd(nc, in_maps, core_ids=list(range(8)))
    LAST_RESULTS = res

    Wkv_np = np.asarray(Wkv, np.float32)
    Wo_np = np.asarray(Wo, np.float32)
    bo_np = np.asarray(bo, np.float32)
    nv_full = np.tile(np.asarray(null_value, np.float32), HEADS)

    out = np.empty((B, N, DIM), np.float32)
    for b in range(B):
        nact = len(idx_i[b])
        if nact:
            s = (res.results[2 * b]["out"][:nact]
                 + res.results[2 * b + 1]["out"][:nact] + bo_np)
            out[b][idx_i[b]] = s
        # masked queries attend uniformly over ALL m+1 positions
        vsum = np.asarray(context[b], np.float32).sum(0) @ Wkv_np[:, INNER:]
        urow = (vsum + nv_full) / (M + 1) @ Wo_np + bo_np
        out[b][~mask_np[b]] = urow
    return out
